# revision 5
# baseline (speedup 1.0000x reference)
"""ALIF neuron rollout (T=100, B=64, J=512, K=1024) on 8 TRN2 NeuronCores.

Strategy (per core, data-parallel over batch, 8 batches/core):
  1. h[t,k] = x[t,:] @ (w_eff*j_eff)  per batch  -> TensorE matmuls, the
     elementwise weight product runs on VectorE over streamed 4MB chunks.
  2. The synaptic-current recurrence (syn_exc/syn_teach share one epsp decay,
     syn_inh is identically zero) is LINEAR, so it folds into TensorE as a
     [T,T] lower-triangular Toeplitz filter:  drive = L1s @ H + L2s @ teacher.
     The filters also bake in dm (=dt/tau_mem), w_teach and a dmc^-t
     rescaling of the whole trajectory.
  3. The nonlinear threshold scan runs in a rescaled domain (state times
     dmc^-t) where the membrane decay becomes identity, so each timestep is
     exactly 4 VectorE instructions + 1 ScalarE instruction over a
     [128 x 64] state tile:
        R  = max(M, 0) + drive_t                (scalar_tensor_tensor)
        sp = R > Btilde                         (tensor_tensor is_gt) = OUTPUT
        M' = sp * (-BIG) + R                    (scalar_tensor_tensor)
        Bd = Btilde * (ab/dmc) + cb*g[t+1]      (ScalarE, in parallel)
        B' = sp * (ab*beta*g[t+1]) + Bd         (scalar_tensor_tensor)
  Layout: k = kt + 8*p (p = partition 0..127, kt = 0..7), state free dim =
  (b*8 + kt) so spikes DMA out k-contiguously per (t, b).
"""
import numpy as np

import concourse.bass as bass
import concourse.tile as tile
from concourse import bacc, mybir
from concourse.bass_utils import run_bass_kernel_spmd

T, B, J, K = 100, 64, 512, 1024
DT = 1.0
NCORES = 8
BLOC = B // NCORES           # 8 batches per core
NKT = 8                      # k interleave factor: k = kt + 8*p
NSLOT = BLOC * NKT           # 64 state slots (free dim of scan tiles)
BIG = 1.0e30
F32 = mybir.dt.float32


def _scalar(v, name):
    v = np.asarray(v, np.float64)
    if v.ndim == 0:
        return float(v)
    if np.ptp(v) != 0.0:
        raise NotImplementedError(f"{name} must be uniform for this kernel")
    return float(v.reshape(-1)[0])


def _host_constants(w_teach, tau_mem, tau_adapt, tau_epsp, thr_0, beta_adapt):
    dm = DT / _scalar(tau_mem, "tau_mem")
    dmc = 1.0 - dm
    da = DT / _scalar(tau_adapt, "tau_adapt")
    ab = 1.0 - da
    thr0 = _scalar(thr_0, "thr_0")
    assert thr0 > 0.0, "kernel assumes thr_0 > 0 (spike compare without relu)"
    cb = da * thr0
    beta = _scalar(beta_adapt, "beta_adapt")
    epsp = 1.0 - DT / _scalar(tau_epsp, "tau_epsp")
    wt = _scalar(w_teach, "w_teach")

    g = dmc ** (-np.arange(T + 1, dtype=np.float64))            # g_t = dmc^-t

    tt_, tau_ = np.meshgrid(np.arange(T), np.arange(T), indexing="ij")
    base = np.where(tau_ <= tt_ - 1,
                    epsp ** np.maximum(tt_ - 1 - tau_, 0), 0.0)
    l1 = (g[:T, None] * dm * base).astype(np.float32)           # [t, tau]
    l2 = (g[:T, None] * dm * wt * base).astype(np.float32)
    l1t = np.ascontiguousarray(l1.T)                            # [tau, t]
    l2t = np.ascontiguousarray(l2.T)

    beta0 = np.full((128, NSLOT), thr0, np.float32)             # btilde_0
    ident = np.eye(T, dtype=np.float32)
    return dict(dm=dm, dmc=dmc, ab=ab, cb=cb, beta=beta, g=g,
                l1t=l1t, l2t=l2t, beta0=beta0, ident=ident)


def build_program(consts):
    """One SPMD program; all 8 cores run it on their own batch shard."""
    g, ab, dmc, cb, beta = (consts["g"], consts["ab"], consts["dmc"],
                            consts["cb"], consts["beta"])
    nc = bacc.Bacc("TRN2", target_bir_lowering=False, debug=False,
                   num_devices=NCORES)

    x_h = nc.declare_dram_parameter("x", [T, BLOC, J], F32, isOutput=False)
    te_h = nc.declare_dram_parameter("teacher", [T, BLOC, K], F32,
                                     isOutput=False)
    we_h = nc.declare_dram_parameter("w_eff", [BLOC, J, K], F32,
                                     isOutput=False)
    je_h = nc.declare_dram_parameter("j_eff", [BLOC, J, K], F32,
                                     isOutput=False)
    l1_h = nc.declare_dram_parameter("l1t", [T, T], F32, isOutput=False)
    l2_h = nc.declare_dram_parameter("l2t", [T, T], F32, isOutput=False)
    id_h = nc.declare_dram_parameter("ident", [T, T], F32, isOutput=False)
    b0_h = nc.declare_dram_parameter("beta0", [128, NSLOT], F32,
                                     isOutput=False)
    out_h = nc.declare_dram_parameter("out", [T, BLOC, K], F32, isOutput=True)

    NCHUNK = 4               # w streamed in chunks of 2 batches (4MB DMAs)
    CB = BLOC // NCHUNK      # batches per chunk = 2
    JT = J // 128            # 4 j-tiles

    from contextlib import ExitStack
    with tile.TileContext(nc) as tc, ExitStack() as ctx:
        cpool = ctx.enter_context(tc.tile_pool(name="consts", bufs=1))
        xtpool = ctx.enter_context(tc.tile_pool(name="xt", bufs=1))
        wpool = ctx.enter_context(tc.tile_pool(name="w", bufs=2))
        tpool = ctx.enter_context(tc.tile_pool(name="teach", bufs=2))
        hpool = ctx.enter_context(tc.tile_pool(name="h", bufs=2))
        dpool = ctx.enter_context(tc.tile_pool(name="drive", bufs=1))
        spool = ctx.enter_context(tc.tile_pool(name="spk", bufs=16))
        scpool = ctx.enter_context(tc.tile_pool(name="scan", bufs=2))
        ps_h = ctx.enter_context(tc.tile_pool(name="psH", bufs=2,
                                              space="PSUM"))
        ps_d = ctx.enter_context(tc.tile_pool(name="psD", bufs=2,
                                              space="PSUM"))
        ps_x = ctx.enter_context(tc.tile_pool(name="psX", bufs=2,
                                              space="PSUM"))

        l1t_sb = cpool.tile([T, T], F32, tag="l1")
        l2t_sb = cpool.tile([T, T], F32, tag="l2")
        id_sb = cpool.tile([T, T], F32, tag="id")
        nc.sync.dma_start(l1t_sb[:], l1_h.ap()[:])
        nc.sync.dma_start(l2t_sb[:], l2_h.ap()[:])
        nc.sync.dma_start(id_sb[:], id_h.ap()[:])

        # --- x -> x^T tiles (PE transpose), [100,128] -> [128,100] each
        # x_sb borrows a weight-pool slot (it is dead before chunk 1 needs it)
        x_sb = wpool.tile([T, BLOC, J], F32, tag="weff")
        nc.sync.dma_start(x_sb[:], x_h.ap()[:])
        xt_sb = xtpool.tile([128, BLOC * JT, T], F32, tag="xt")
        for b in range(BLOC):
            for jt in range(JT):
                xp = ps_x.tile([128, T], F32, tag="xps")
                nc.tensor.transpose(
                    xp[:], x_sb[:, b, jt * 128:(jt + 1) * 128], id_sb[:])
                nc.scalar.copy(xt_sb[:, b * JT + jt, :], xp[:])

        # --- drive tiles: [128 (p), 64 (b*8+kt), 100 (t)]
        drive_sb = dpool.tile([128, NSLOT, T], F32, tag="drive")

        for c in range(NCHUNK):
            bsl = slice(c * CB, (c + 1) * CB)
            we_t = wpool.tile([128, CB, JT, K], F32, tag="weff")
            je_t = wpool.tile([128, CB, JT, K], F32, tag="jeff")
            te_t = tpool.tile([T, CB, K], F32, tag="teach")
            nc.sync.dma_start(
                we_t[:], we_h.ap()[bsl].rearrange("b (jt p) k -> p b jt k",
                                                  p=128))
            nc.sync.dma_start(
                je_t[:], je_h.ap()[bsl].rearrange("b (jt p) k -> p b jt k",
                                                  p=128))
            nc.sync.dma_start(te_t[:], te_h.ap()[:, bsl, :])

            # w = w_eff * j_eff, in place (one [128, 8192] f32 DVE op)
            wf = we_t[:].rearrange("p b jt k -> p (b jt k)")
            jf = je_t[:].rearrange("p b jt k -> p (b jt k)")
            nc.vector.tensor_tensor(wf, wf, jf, mybir.AluOpType.mult)

            for i in range(CB):
                b = c * CB + i
                hps = ps_h.tile([T, K], F32, tag="hps")
                for jt in range(JT):
                    for half in range(2):
                        nc.tensor.matmul(
                            hps[:, half * 512:(half + 1) * 512],
                            lhsT=xt_sb[:, b * JT + jt, :],
                            rhs=we_t[:, i, jt, half * 512:(half + 1) * 512],
                            start=(jt == 0), stop=(jt == JT - 1))
                hsb = hpool.tile([T, K], F32, tag="hsb")
                nc.scalar.copy(hsb[:], hps[:])

                # drive^T[k,t] per kt:  H[tau, k-slice] @ L1s^T  (+ teacher)
                hkt = hsb[:].rearrange("t (p kt) -> t kt p", kt=NKT)
                tkt = te_t[:, i, :].rearrange("t (p kt) -> t kt p", kt=NKT)
                for kt in range(NKT):
                    dps = ps_d.tile([128, T], F32, tag="dps")
                    nc.tensor.matmul(dps[:], lhsT=hkt[:, kt, :],
                                     rhs=l1t_sb[:], start=True, stop=False)
                    nc.tensor.matmul(dps[:], lhsT=tkt[:, kt, :],
                                     rhs=l2t_sb[:], start=False, stop=True)
                    nc.scalar.copy(drive_sb[:, b * NKT + kt, :], dps[:])

        # --- the sequential threshold scan ---
        m_prev = scpool.tile([128, NSLOT], F32, tag="M")
        b_prev = scpool.tile([128, NSLOT], F32, tag="B")
        nc.vector.memset(m_prev[:], 0.0)
        nc.sync.dma_start(b_prev[:], b0_h.ap()[:])
        out_r = out_h.ap().rearrange("t b (p kt) -> t p b kt", kt=NKT)
        for t in range(T):
            rt = scpool.tile([128, NSLOT], F32, tag="R")
            nc.vector.scalar_tensor_tensor(
                rt[:], m_prev[:], 0.0, drive_sb[:, :, t],
                op0=mybir.AluOpType.max, op1=mybir.AluOpType.add)
            spk = spool.tile([128, NSLOT], F32, tag="spk")
            nc.vector.tensor_tensor(spk[:], rt[:], b_prev[:],
                                    mybir.AluOpType.is_gt)
            m_new = scpool.tile([128, NSLOT], F32, tag="M")
            nc.vector.scalar_tensor_tensor(
                m_new[:], spk[:], -BIG, rt[:],
                op0=mybir.AluOpType.mult, op1=mybir.AluOpType.add)
            b_dec = scpool.tile([128, NSLOT], F32, tag="Bd")
            nc.scalar.activation(b_dec[:], b_prev[:],
                                 mybir.ActivationFunctionType.Copy,
                                 bias=float(cb * g[t + 1]),
                                 scale=float(ab / dmc))
            b_new = scpool.tile([128, NSLOT], F32, tag="B")
            nc.vector.scalar_tensor_tensor(
                b_new[:], spk[:], float(ab * beta * g[t + 1]), b_dec[:],
                op0=mybir.AluOpType.mult, op1=mybir.AluOpType.add)
            nc.sync.dma_start(
                out_r[t], spk[:].rearrange("p (b kt) -> p b kt", kt=NKT))
            m_prev, b_prev = m_new, b_new

    nc.compile()
    return nc


def _prepare(inputs):
    x = np.ascontiguousarray(np.asarray(inputs["x"], np.float32))
    teacher = np.ascontiguousarray(np.asarray(inputs["teacher"], np.float32))
    w_eff = np.ascontiguousarray(np.asarray(inputs["w_eff"], np.float32))
    j_eff = np.ascontiguousarray(np.asarray(inputs["j_eff"], np.float32))
    consts = _host_constants(
        inputs["w_teach"], inputs["tau_mem"], inputs["tau_adapt"],
        inputs["tau_epsp"], inputs["thr_0"], inputs["beta_adapt"])
    in_maps = []
    for i in range(NCORES):
        sl = slice(i * BLOC, (i + 1) * BLOC)
        in_maps.append({
            "x": np.ascontiguousarray(x[:, sl]),
            "teacher": np.ascontiguousarray(teacher[:, sl]),
            "w_eff": np.ascontiguousarray(w_eff[sl]),
            "j_eff": np.ascontiguousarray(j_eff[sl]),
            "l1t": consts["l1t"], "l2t": consts["l2t"],
            "ident": consts["ident"], "beta0": consts["beta0"],
        })
    return consts, in_maps


def run(inputs, trace=False, **kw):
    consts, in_maps = _prepare(inputs)
    nc = build_program(consts)
    res = run_bass_kernel_spmd(nc, in_maps, core_ids=list(range(NCORES)),
                               trace=trace, **kw)
    out = np.concatenate([res.results[i]["out"] for i in range(NCORES)],
                         axis=1)
    return out.astype(np.float32), res


def kernel(**inputs) -> np.ndarray:
    out, _ = run(inputs)
    return out


# revision 7
# speedup vs baseline: 1.0747x; 1.0747x over previous
"""ALIF neuron rollout (T=100, B=64, J=512, K=1024) on 8 TRN2 NeuronCores.

Strategy (per core, data-parallel over batch, 8 batches/core):
  1. h[t,k] = x[t,:] @ (w_eff*j_eff)  per batch  -> TensorE matmuls, the
     elementwise weight product runs on VectorE over streamed 4MB chunks.
  2. The synaptic-current recurrence (syn_exc/syn_teach share one epsp decay,
     syn_inh is identically zero) is LINEAR, so it folds into TensorE as a
     [T,T] lower-triangular Toeplitz filter:  drive = L1s @ H + L2s @ teacher.
     The filters also bake in dm (=dt/tau_mem), w_teach and a dmc^-t
     rescaling of the whole trajectory.
  3. The nonlinear threshold scan runs in a rescaled domain (state times
     dmc^-t) where the membrane decay becomes identity, so each timestep is
     exactly 4 VectorE instructions + 1 ScalarE instruction over a
     [128 x 64] state tile:
        R  = max(M, 0) + drive_t                (scalar_tensor_tensor)
        sp = R > Btilde                         (tensor_tensor is_gt) = OUTPUT
        M' = sp * (-BIG) + R                    (scalar_tensor_tensor)
        Bd = Btilde * (ab/dmc) + cb*g[t+1]      (ScalarE, in parallel)
        B' = sp * (ab*beta*g[t+1]) + Bd         (scalar_tensor_tensor)
  Layout: k = kt + 8*p (p = partition 0..127, kt = 0..7), state free dim =
  (b*8 + kt) so spikes DMA out k-contiguously per (t, b).
"""
import numpy as np

import concourse.bass as bass
import concourse.tile as tile
from concourse import bacc, mybir
from concourse.bass_utils import run_bass_kernel_spmd

T, B, J, K = 100, 64, 512, 1024
DT = 1.0
NCORES = 8
BLOC = B // NCORES           # 8 batches per core
NKT = 8                      # k interleave factor: k = kt + 8*p
NSLOT = BLOC * NKT           # 64 state slots (free dim of scan tiles)
BIG = 1.0e30
F32 = mybir.dt.float32


def _scalar(v, name):
    v = np.asarray(v, np.float64)
    if v.ndim == 0:
        return float(v)
    if np.ptp(v) != 0.0:
        raise NotImplementedError(f"{name} must be uniform for this kernel")
    return float(v.reshape(-1)[0])


def _host_constants(w_teach, tau_mem, tau_adapt, tau_epsp, thr_0, beta_adapt):
    dm = DT / _scalar(tau_mem, "tau_mem")
    dmc = 1.0 - dm
    da = DT / _scalar(tau_adapt, "tau_adapt")
    ab = 1.0 - da
    thr0 = _scalar(thr_0, "thr_0")
    assert thr0 > 0.0, "kernel assumes thr_0 > 0 (spike compare without relu)"
    cb = da * thr0
    beta = _scalar(beta_adapt, "beta_adapt")
    epsp = 1.0 - DT / _scalar(tau_epsp, "tau_epsp")
    wt = _scalar(w_teach, "w_teach")

    g = dmc ** (-np.arange(T + 1, dtype=np.float64))            # g_t = dmc^-t

    tt_, tau_ = np.meshgrid(np.arange(T), np.arange(T), indexing="ij")
    base = np.where(tau_ <= tt_ - 1,
                    epsp ** np.maximum(tt_ - 1 - tau_, 0), 0.0)
    l1 = (g[:T, None] * dm * base).astype(np.float32)           # [t, tau]
    l2 = (g[:T, None] * dm * wt * base).astype(np.float32)
    l1t = np.ascontiguousarray(l1.T)                            # [tau, t]
    l2t = np.ascontiguousarray(l2.T)

    beta0 = np.full((128, NSLOT), thr0, np.float32)             # btilde_0
    ident = np.eye(T, dtype=np.float32)
    return dict(dm=dm, dmc=dmc, ab=ab, cb=cb, beta=beta, g=g,
                l1t=l1t, l2t=l2t, beta0=beta0, ident=ident)


def build_program(consts):
    """One SPMD program; all 8 cores run it on their own batch shard."""
    g, ab, dmc, cb, beta = (consts["g"], consts["ab"], consts["dmc"],
                            consts["cb"], consts["beta"])
    nc = bacc.Bacc("TRN2", target_bir_lowering=False, debug=False,
                   num_devices=NCORES)

    x_h = nc.declare_dram_parameter("x", [T, BLOC, J], F32, isOutput=False)
    te_h = nc.declare_dram_parameter("teacher", [T, BLOC, K], F32,
                                     isOutput=False)
    we_h = nc.declare_dram_parameter("w_eff", [BLOC, J, K], F32,
                                     isOutput=False)
    je_h = nc.declare_dram_parameter("j_eff", [BLOC, J, K], F32,
                                     isOutput=False)
    l1_h = nc.declare_dram_parameter("l1t", [T, T], F32, isOutput=False)
    l2_h = nc.declare_dram_parameter("l2t", [T, T], F32, isOutput=False)
    id_h = nc.declare_dram_parameter("ident", [T, T], F32, isOutput=False)
    b0_h = nc.declare_dram_parameter("beta0", [128, NSLOT], F32,
                                     isOutput=False)
    out_h = nc.declare_dram_parameter("out", [T, BLOC, K], F32, isOutput=True)

    NCHUNK = 4               # w streamed in chunks of 2 batches (4MB DMAs)
    CB = BLOC // NCHUNK      # batches per chunk = 2
    JT = J // 128            # 4 j-tiles

    from contextlib import ExitStack
    with tile.TileContext(nc) as tc, ExitStack() as ctx:
        cpool = ctx.enter_context(tc.tile_pool(name="consts", bufs=1))
        xtpool = ctx.enter_context(tc.tile_pool(name="xt", bufs=1))
        wpool = ctx.enter_context(tc.tile_pool(name="w", bufs=2))
        tpool = ctx.enter_context(tc.tile_pool(name="teach", bufs=2))
        hpool = ctx.enter_context(tc.tile_pool(name="h", bufs=2))
        dpool = ctx.enter_context(tc.tile_pool(name="drive", bufs=1))
        spool = ctx.enter_context(tc.tile_pool(name="spk", bufs=16))
        scpool = ctx.enter_context(tc.tile_pool(name="scan", bufs=2))
        ps_h = ctx.enter_context(tc.tile_pool(name="psH", bufs=2,
                                              space="PSUM"))
        ps_d = ctx.enter_context(tc.tile_pool(name="psD", bufs=2,
                                              space="PSUM"))
        ps_x = ctx.enter_context(tc.tile_pool(name="psX", bufs=2,
                                              space="PSUM"))

        l1t_sb = cpool.tile([T, T], F32, tag="l1")
        l2t_sb = cpool.tile([T, T], F32, tag="l2")
        id_sb = cpool.tile([T, T], F32, tag="id")
        nc.sync.dma_start(l1t_sb[:], l1_h.ap()[:])
        nc.sync.dma_start(l2t_sb[:], l2_h.ap()[:])
        nc.sync.dma_start(id_sb[:], id_h.ap()[:])

        # --- x-filter fold: xf[j,t] = sum_tau x[tau,j] * L1s[t,tau]
        # (one matmul per (b,jt): lhsT = x tile in natural layout, rhs = L1s^T)
        # x_sb borrows a weight-pool slot (it is dead before chunk 1 needs it)
        x_sb = wpool.tile([T, BLOC, J], F32, tag="weff")
        nc.sync.dma_start(x_sb[:], x_h.ap()[:])
        xt_sb = xtpool.tile([128, BLOC * JT, T], F32, tag="xt")
        for b in range(BLOC):
            for jt in range(JT):
                xp = ps_x.tile([128, T], F32, tag="xps")
                nc.tensor.matmul(xp[:],
                                 lhsT=x_sb[:, b, jt * 128:(jt + 1) * 128],
                                 rhs=l1t_sb[:], start=True, stop=True)
                nc.scalar.copy(xt_sb[:, b * JT + jt, :], xp[:])

        # --- drive tiles: [128 (p), 64 (b*8+kt), 100 (t)]
        drive_sb = dpool.tile([128, NSLOT, T], F32, tag="drive")

        for c in range(NCHUNK):
            bsl = slice(c * CB, (c + 1) * CB)
            we_t = wpool.tile([128, CB, JT, K], F32, tag="weff")
            je_t = wpool.tile([128, CB, JT, K], F32, tag="jeff")
            te_t = tpool.tile([T, CB, K], F32, tag="teach")
            nc.sync.dma_start(
                we_t[:], we_h.ap()[bsl].rearrange("b (jt p) k -> p b jt k",
                                                  p=128))
            nc.sync.dma_start(
                je_t[:], je_h.ap()[bsl].rearrange("b (jt p) k -> p b jt k",
                                                  p=128))
            nc.sync.dma_start(te_t[:], te_h.ap()[:, bsl, :])

            # w = w_eff * j_eff, in place (one [128, 8192] f32 DVE op)
            wf = we_t[:].rearrange("p b jt k -> p (b jt k)")
            jf = je_t[:].rearrange("p b jt k -> p (b jt k)")
            nc.vector.tensor_tensor(wf, wf, jf, mybir.AluOpType.mult)

            for i in range(CB):
                b = c * CB + i
                # S[t,k] = sum_j xf[j,t]*w[j,k] + sum_tau L2s[t,tau]*teach[tau,k]
                hps = ps_h.tile([T, K], F32, tag="hps")
                for half in range(2):
                    ksl = slice(half * 512, (half + 1) * 512)
                    for jt in range(JT):
                        nc.tensor.matmul(
                            hps[:, ksl],
                            lhsT=xt_sb[:, b * JT + jt, :],
                            rhs=we_t[:, i, jt, ksl],
                            start=(jt == 0), stop=False)
                    nc.tensor.matmul(hps[:, ksl], lhsT=l2t_sb[:],
                                     rhs=te_t[:, i, ksl],
                                     start=False, stop=True)
                hsb = hpool.tile([T, K], F32, tag="hsb")
                nc.scalar.copy(hsb[:], hps[:])

                # transpose drive [t,k] -> [k,t] per kt slice (k = kt + 8*p)
                skt = hsb[:].rearrange("t (p kt) -> t kt p", kt=NKT)
                for kt in range(NKT):
                    dps = ps_d.tile([128, T], F32, tag="dps")
                    nc.tensor.transpose(dps[:], skt[:, kt, :], id_sb[:])
                    nc.scalar.copy(drive_sb[:, b * NKT + kt, :], dps[:])

        # --- the sequential threshold scan ---
        m_prev = scpool.tile([128, NSLOT], F32, tag="M")
        b_prev = scpool.tile([128, NSLOT], F32, tag="B")
        nc.vector.memset(m_prev[:], 0.0)
        nc.sync.dma_start(b_prev[:], b0_h.ap()[:])
        out_r = out_h.ap().rearrange("t b (p kt) -> t p b kt", kt=NKT)
        for t in range(T):
            rt = scpool.tile([128, NSLOT], F32, tag="R")
            nc.vector.scalar_tensor_tensor(
                rt[:], m_prev[:], 0.0, drive_sb[:, :, t],
                op0=mybir.AluOpType.max, op1=mybir.AluOpType.add)
            spk = spool.tile([128, NSLOT], F32, tag="spk")
            nc.vector.tensor_tensor(spk[:], rt[:], b_prev[:],
                                    mybir.AluOpType.is_gt)
            m_new = scpool.tile([128, NSLOT], F32, tag="M")
            nc.vector.scalar_tensor_tensor(
                m_new[:], spk[:], -BIG, rt[:],
                op0=mybir.AluOpType.mult, op1=mybir.AluOpType.add)
            b_dec = scpool.tile([128, NSLOT], F32, tag="Bd")
            nc.scalar.activation(b_dec[:], b_prev[:],
                                 mybir.ActivationFunctionType.Copy,
                                 bias=float(cb * g[t + 1]),
                                 scale=float(ab / dmc))
            b_new = scpool.tile([128, NSLOT], F32, tag="B")
            nc.vector.scalar_tensor_tensor(
                b_new[:], spk[:], float(ab * beta * g[t + 1]), b_dec[:],
                op0=mybir.AluOpType.mult, op1=mybir.AluOpType.add)
            nc.sync.dma_start(
                out_r[t], spk[:].rearrange("p (b kt) -> p b kt", kt=NKT))
            m_prev, b_prev = m_new, b_new

    nc.compile()
    return nc


def _prepare(inputs):
    x = np.ascontiguousarray(np.asarray(inputs["x"], np.float32))
    teacher = np.ascontiguousarray(np.asarray(inputs["teacher"], np.float32))
    w_eff = np.ascontiguousarray(np.asarray(inputs["w_eff"], np.float32))
    j_eff = np.ascontiguousarray(np.asarray(inputs["j_eff"], np.float32))
    consts = _host_constants(
        inputs["w_teach"], inputs["tau_mem"], inputs["tau_adapt"],
        inputs["tau_epsp"], inputs["thr_0"], inputs["beta_adapt"])
    in_maps = []
    for i in range(NCORES):
        sl = slice(i * BLOC, (i + 1) * BLOC)
        in_maps.append({
            "x": np.ascontiguousarray(x[:, sl]),
            "teacher": np.ascontiguousarray(teacher[:, sl]),
            "w_eff": np.ascontiguousarray(w_eff[sl]),
            "j_eff": np.ascontiguousarray(j_eff[sl]),
            "l1t": consts["l1t"], "l2t": consts["l2t"],
            "ident": consts["ident"], "beta0": consts["beta0"],
        })
    return consts, in_maps


def run(inputs, trace=False, **kw):
    consts, in_maps = _prepare(inputs)
    nc = build_program(consts)
    res = run_bass_kernel_spmd(nc, in_maps, core_ids=list(range(NCORES)),
                               trace=trace, **kw)
    out = np.concatenate([res.results[i]["out"] for i in range(NCORES)],
                         axis=1)
    return out.astype(np.float32), res


def kernel(**inputs) -> np.ndarray:
    out, _ = run(inputs)
    return out
